# revision 1
# baseline (speedup 1.0000x reference)
"""Trainium2 Bass kernel for nn_ChimeraV2Block (dual-softmax differential
sliding-window attention block, B=1 S=2048 D=2048, 16 q-heads / 4 kv-heads,
head_dim 128, window 512).

Sharding: tensor-parallel over heads across 8 NeuronCores. Core c owns
q-heads {2c, 2c+1} and kv-head c//2 (GQA groups align with the split).
Wq/Wk/Wv column-sharded, Wo row-sharded; the 8 fp32 partial outputs are
summed on the host (the "all-reduce").
"""

import sys

if "/opt/trn_rl_repo" not in sys.path:
    sys.path.insert(0, "/opt/trn_rl_repo")

import numpy as np
import ml_dtypes

BF = ml_dtypes.bfloat16

S = 2048
D = 2048
H = 16
HK = 4
HD = 128
WIN = 512
THETA = 10000.0
N_CORES = 8
NQT = S // 128          # 16 q row-tiles
NKT = D // 128          # 16 contraction tiles for the projections
WMAX = WIN + 128        # 640: max key-window width per q-tile
NEG = -1.0e30

_CACHE = {}


def _tables():
    """RoPE tables [128, S] fp32 with head-dim-duplicated frequencies
    (row p uses invf[p % 64]), so every op reads the table at the same
    base partition as its (possibly swapped) q operand. Q tables are
    pre-scaled by the attention scale 1/sqrt(64)."""
    invf = 1.0 / (THETA ** (np.arange(0, HD, 2, dtype=np.float64) / HD))  # [64]
    t = np.arange(S, dtype=np.float64)
    fr = np.outer(invf, t)  # [64, S]
    cosf = np.concatenate([np.cos(fr)] * 2, axis=0)
    sinf = np.concatenate([np.sin(fr)] * 2, axis=0)
    return (np.ascontiguousarray(cosf * 0.125, dtype=np.float32),
            np.ascontiguousarray(sinf * 0.125, dtype=np.float32),
            np.ascontiguousarray(cosf, dtype=np.float32),
            np.ascontiguousarray(sinf, dtype=np.float32))


def _masks():
    p = np.arange(128)[:, None]
    c = np.arange(WMAX)[None, :]
    band = (c - p >= 1) & (c - p <= WIN)
    mw = np.where(band, 0.0, NEG).astype(BF)          # [128, 640]
    cc = np.arange(128)[None, :]
    mc = np.where(cc <= p, 0.0, NEG).astype(BF)       # [128, 128] causal
    # edge mask: cols [0,512) allowed, cols [512,640) causal triangle.
    # slicing the last w cols gives the mask for edge q-tiles (qi < 4).
    me = np.zeros((128, WMAX), dtype=BF)
    me[:, WIN:] = mc
    return mw, me


def _build_program():
    import concourse.bacc as bacc
    import concourse.tile as tile
    from concourse import mybir

    bf = mybir.dt.bfloat16
    f32 = mybir.dt.float32
    EXP = mybir.ActivationFunctionType.Exp
    MULT = mybir.AluOpType.mult
    ADD = mybir.AluOpType.add
    MAX = mybir.AluOpType.max
    DIV = mybir.AluOpType.divide

    nc = bacc.Bacc("TRN2", target_bir_lowering=False, debug=False,
                   num_devices=N_CORES)

    xt_d = nc.dram_tensor("xt", [128, NKT, S], bf, kind="ExternalInput")
    wq_d = nc.dram_tensor("wq", [128, NKT, 2, 128], bf, kind="ExternalInput")
    wk_d = nc.dram_tensor("wk", [128, NKT, 128], bf, kind="ExternalInput")
    wv_d = nc.dram_tensor("wv", [128, NKT, 128], bf, kind="ExternalInput")
    wo_d = nc.dram_tensor("wo", [128, 2, D], bf, kind="ExternalInput")
    lamn_d = nc.dram_tensor("lamn", [1, 2], f32, kind="ExternalInput")
    f16 = mybir.dt.float16
    out_d = nc.dram_tensor("outp", [S, D], f16, kind="ExternalOutput")

    tqc_np, tqs_np, tkc_np, tks_np = _tables()
    mw_np, me_np = _masks()
    tqc_d = nc.inline_tensor(tqc_np, "tab_qc")
    tqs_d = nc.inline_tensor(tqs_np, "tab_qs")
    tkc_d = nc.inline_tensor(tkc_np, "tab_kc")
    tks_d = nc.inline_tensor(tks_np, "tab_ks")
    mw_d = nc.inline_tensor(mw_np, "mask_win")
    me_d = nc.inline_tensor(me_np, "mask_edge")
    idb_d = nc.inline_tensor(np.eye(128, dtype=BF), "ident_bf")
    idf_d = nc.inline_tensor(np.eye(128, dtype=np.float32), "ident_f32")

    with tile.TileContext(nc) as tc:
        with tc.tile_pool(name="xpool", bufs=1) as xp, \
             tc.tile_pool(name="wpool", bufs=1) as wp, \
             tc.tile_pool(name="pers", bufs=1) as pers:

            xt = xp.tile([128, NKT, S], bf)
            for kti in range(NKT):
                for hh in range(2):
                    sl = slice(hh * (S // 2), (hh + 1) * (S // 2))
                    nc.sync.dma_start(out=xt[:, kti, sl], in_=xt_d[:, kti, sl])
            wq = wp.tile([128, NKT, 2, 128], bf)
            for i in range(4):
                nc.sync.dma_start(out=wq[:, 4 * i:4 * i + 4], in_=wq_d[:, 4 * i:4 * i + 4])
            wk = wp.tile([128, NKT, 128], bf)
            nc.sync.dma_start(out=wk[:, 0:8], in_=wk_d[:, 0:8])
            nc.sync.dma_start(out=wk[:, 8:16], in_=wk_d[:, 8:16])
            wv = wp.tile([128, NKT, 128], bf)
            nc.sync.dma_start(out=wv[:, 0:8], in_=wv_d[:, 0:8])
            nc.sync.dma_start(out=wv[:, 8:16], in_=wv_d[:, 8:16])
            wo = wp.tile([128, 2, D], bf)
            for i in range(4):
                nc.sync.dma_start(out=wo[:, :, 512 * i:512 * (i + 1)],
                                  in_=wo_d[:, :, 512 * i:512 * (i + 1)])
            tqc = wp.tile([128, S], f32)
            tqs = wp.tile([128, S], f32)
            tkc = wp.tile([128, S], f32)
            tks = wp.tile([128, S], f32)
            for i in range(4):
                sl = slice(512 * i, 512 * (i + 1))
                nc.sync.dma_start(out=tqc[:, sl], in_=tqc_d[:, sl])
                nc.sync.dma_start(out=tqs[:, sl], in_=tqs_d[:, sl])
                nc.sync.dma_start(out=tkc[:, sl], in_=tkc_d[:, sl])
                nc.sync.dma_start(out=tks[:, sl], in_=tks_d[:, sl])
            mw = wp.tile([128, WMAX], bf)
            nc.sync.dma_start(out=mw[:], in_=mw_d[:])
            me = wp.tile([128, WMAX], bf)
            nc.sync.dma_start(out=me[:], in_=me_d[:])
            idb = wp.tile([128, 128], bf)
            nc.sync.dma_start(out=idb[:], in_=idb_d[:])
            lamn = wp.tile([1, 2], f32)
            nc.sync.dma_start(out=lamn[:], in_=lamn_d[:])
            lamb = wp.tile([128, 2], f32)
            nc.gpsimd.partition_broadcast(lamb[:], lamn[:])

            qt = pers.tile([128, 2, S], bf)      # RoPE'd scaled q, hd-major
            kt = pers.tile([128, S], bf)         # RoPE'd k, hd-major
            vsm = pers.tile([128, NQT, 128], bf)  # v, S-major [s, hd]
            att = pers.tile([128, 2, S], bf)     # attention out^T, hd-major

            # ---- Phase 1: projections + RoPE + v transpose ----
            with tc.tile_pool(name="pp", bufs=1, space="PSUM") as pp, \
                 tc.tile_pool(name="pt", bufs=2) as pt:
                for nch in range(4):
                    sl = slice(nch * 512, (nch + 1) * 512)
                    ps_q0 = pp.tile([128, 512], f32, tag="pq0", bufs=2)
                    ps_q1 = pp.tile([128, 512], f32, tag="pq1", bufs=2)
                    ps_k = pp.tile([128, 512], f32, tag="pk", bufs=1)
                    ps_v = pp.tile([128, 512], f32, tag="pv", bufs=1)
                    for kti in range(NKT):
                        st = kti == 0
                        sp = kti == NKT - 1
                        rhs = xt[:, kti, sl]
                        nc.tensor.matmul(ps_q0[:], wq[:, kti, 0, :], rhs, start=st, stop=sp)
                        nc.tensor.matmul(ps_q1[:], wq[:, kti, 1, :], rhs, start=st, stop=sp)
                        nc.tensor.matmul(ps_k[:], wk[:, kti, :], rhs, start=st, stop=sp)
                        nc.tensor.matmul(ps_v[:], wv[:, kti, :], rhs, start=st, stop=sp)
                    for ps, outt, tabc, tabs in (
                            (ps_q0, qt[:, 0, sl], tqc, tqs),
                            (ps_q1, qt[:, 1, sl], tqc, tqs),
                            (ps_k, kt[:, sl], tkc, tks)):
                        f = pt.tile([128, 512], f32, tag="f")
                        m1 = pt.tile([128, 512], f32, tag="m1")
                        m2 = pt.tile([128, 512], f32, tag="m2")
                        nc.vector.tensor_copy(out=f[:], in_=ps[:])
                        # m2 = rotate_half partner * sin
                        nc.vector.tensor_mul(m2[0:64, :], f[64:128, :], tabs[64:128, sl])
                        nc.vector.tensor_mul(m2[64:128, :], f[0:64, :], tabs[0:64, sl])
                        nc.vector.tensor_mul(m1[:], f[:], tabc[:, sl])
                        nc.vector.tensor_sub(outt[0:64, :], m1[0:64, :], m2[0:64, :])
                        nc.vector.tensor_add(outt[64:128, :], m1[64:128, :], m2[64:128, :])
                    vtmp = pt.tile([128, 512], bf, tag="vtmp")
                    nc.vector.tensor_copy(out=vtmp[:], in_=ps_v[:])
                    ps_tv = pp.tile([128, 4, 128], bf, tag="ptv", bufs=2)
                    for j in range(4):
                        nc.tensor.transpose(ps_tv[:, j, :], vtmp[:, 128 * j:128 * (j + 1)], idb[:])
                    nc.vector.tensor_copy(out=vsm[:, 4 * nch:4 * (nch + 1), :], in_=ps_tv[:])

            # ---- Phase 2: attention ----
            with tc.tile_pool(name="psc", bufs=1, space="PSUM") as psc, \
                 tc.tile_pool(name="pse", bufs=1) as pse, \
                 tc.tile_pool(name="psm", bufs=1) as psm:
                for qi in range(NQT):
                    qsl = slice(qi * 128, (qi + 1) * 128)
                    kw = min(qi + 1, 5)
                    w = kw * 128
                    kstart = max(0, qi - 4)
                    kwin = slice(kstart * 128, kstart * 128 + w)

                    ps_av0 = psc.tile([128, 128], f32, tag="av0", bufs=1)
                    ps_av1 = psc.tile([128, 128], f32, tag="av1", bufs=1)
                    ps_av = [ps_av0, ps_av1]

                    for h in range(2):
                        ps_s1 = psc.tile([128, WMAX], f32, tag="s", bufs=2)
                        ps_s2 = psc.tile([128, WMAX], f32, tag="s", bufs=2)
                        for half, ps in ((0, ps_s1), (1, ps_s2)):
                            hp = slice(64 * half, 64 * half + 64)
                            lhs = qt[hp, h, qsl]
                            msk = mw if qi >= 4 else me[:, WMAX - w:WMAX]
                            wa = min(w, 512)
                            nc.tensor.matmul(ps[:, 0:wa], idb[:], msk[:, 0:wa],
                                             start=True, stop=False)
                            nc.tensor.matmul(ps[:, 0:wa], lhs,
                                             kt[hp, kwin][:, 0:wa],
                                             start=False, stop=True)
                            if w > 512:
                                nc.tensor.matmul(ps[:, 512:w], idb[:],
                                                 msk[:, 512:w],
                                                 start=True, stop=False)
                                nc.tensor.matmul(ps[:, 512:w], lhs,
                                                 kt[hp, kwin][:, 512:w],
                                                 start=False, stop=True)

                        e1 = pse.tile([128, WMAX], bf, tag="e1", bufs=2)
                        e2 = pse.tile([128, WMAX], bf, tag="e2", bufs=2)
                        s1 = psm.tile([128, 1], f32, tag="s1", bufs=4)
                        s2 = psm.tile([128, 1], f32, tag="s2", bufs=4)
                        nc.scalar.activation(out=e1[:, 0:w], in_=ps_s1[:, 0:w],
                                             func=EXP, accum_out=s1[:])
                        nc.scalar.activation(out=e2[:, 0:w], in_=ps_s2[:, 0:w],
                                             func=EXP, accum_out=s2[:])

                        # cneg = -(lam * s1 / s2)   (lamn holds -lam)
                        r2 = psm.tile([128, 1], f32, tag="r2", bufs=4)
                        nc.vector.reciprocal(out=r2[:], in_=s2[:])
                        cneg = psm.tile([128, 1], f32, tag="cneg", bufs=4)
                        nc.vector.scalar_tensor_tensor(
                            out=cneg[:], in0=s1[:], scalar=lamb[:, h:h + 1],
                            in1=r2[:], op0=MULT, op1=MULT)
                        # g0 = e1 + cneg*e2 ; g = relu(g0), accum D'
                        g0 = pse.tile([128, WMAX], bf, tag="g0", bufs=2)
                        g = pse.tile([128, WMAX], bf, tag="g", bufs=2)
                        dsum = psm.tile([128, 1], f32, tag="dsum", bufs=4)
                        nc.vector.scalar_tensor_tensor(
                            out=g0[:, 0:w], in0=e2[:, 0:w], scalar=cneg[:],
                            in1=e1[:, 0:w], op0=MULT, op1=ADD)
                        nc.vector.tensor_scalar(
                            out=g[:, 0:w], in0=g0[:, 0:w], scalar1=0.0,
                            scalar2=0.0, op0=MAX, op1=ADD, accum_out=dsum[:])
                        # recd = 1 / (D' + 1e-6 * s1); gn = g * recd
                        dtmp = psm.tile([128, 1], f32, tag="dtmp", bufs=4)
                        nc.vector.scalar_tensor_tensor(
                            out=dtmp[:], in0=s1[:], scalar=1e-6, in1=dsum[:],
                            op0=MULT, op1=ADD)
                        recd = psm.tile([128, 1], f32, tag="recd", bufs=4)
                        nc.vector.reciprocal(out=recd[:], in_=dtmp[:])
                        gn = pse.tile([128, WMAX], bf, tag="gn", bufs=2)
                        nc.vector.tensor_scalar(
                            out=gn[:, 0:w], in0=g[:, 0:w], scalar1=recd[:],
                            scalar2=0.0, op0=MULT, op1=ADD)

                        # transpose gn -> gT (PSUM) -> SBUF
                        ps_tr = psc.tile([128, kw, 128], bf, tag="trg", bufs=2)
                        for j in range(kw):
                            nc.tensor.transpose(ps_tr[:, j, :],
                                                gn[:, 128 * j:128 * (j + 1)], idb[:])
                        gts = pse.tile([128, 5, 128], bf, tag="gts", bufs=2)
                        nc.vector.tensor_copy(out=gts[:, 0:kw, :], in_=ps_tr[:])

                        # AV: out^T[hd, q] += v[k,:].T-contract over window
                        for j in range(kw):
                            nc.tensor.matmul(ps_av[h][:], vsm[:, kstart + j, :],
                                             gts[:, j, :],
                                             start=(j == 0), stop=(j == kw - 1))

                    nc.vector.tensor_copy(out=att[:, 0, qsl], in_=ps_av[0][:])
                    nc.vector.tensor_copy(out=att[:, 1, qsl], in_=ps_av[1][:])

            # ---- Phase 3: output projection (row-sharded Wo partial) ----
            with tc.tile_pool(name="po", bufs=1, space="PSUM") as po, \
                 tc.tile_pool(name="pos", bufs=1) as pos:
                for qi in range(NQT):
                    qsl = slice(qi * 128, (qi + 1) * 128)
                    for dch in range(4):
                        dsl = slice(dch * 512, (dch + 1) * 512)
                        ps_o = po.tile([128, 512], f32, tag="o", bufs=5)
                        nc.tensor.matmul(ps_o[:], att[:, 0, qsl], wo[:, 0, dsl],
                                         start=True, stop=False)
                        nc.tensor.matmul(ps_o[:], att[:, 1, qsl], wo[:, 1, dsl],
                                         start=False, stop=True)
                        so = pos.tile([128, 512], f16, tag="so", bufs=6)
                        nc.scalar.copy(out=so[:], in_=ps_o[:])
                        nc.sync.dma_start(out=out_d[qsl, dsl], in_=so[:])

    nc.compile()
    return nc


def get_program():
    if "nc" not in _CACHE:
        _CACHE["nc"] = _build_program()
    return _CACHE["nc"]


def _prep_inputs(x, Wq, Wk, Wv, Wo, lam):
    xt = np.ascontiguousarray(x.reshape(S, D).T.astype(BF)
                              .reshape(NKT, 128, S).transpose(1, 0, 2))
    in_maps = []
    for c in range(N_CORES):
        h0 = 2 * c
        kv = c // 2
        wq_c = np.ascontiguousarray(
            Wq[:, h0 * 128:(h0 + 2) * 128].astype(BF)
            .reshape(NKT, 128, 2, 128).transpose(1, 0, 2, 3))
        wk_c = np.ascontiguousarray(
            Wk[:, kv * 128:(kv + 1) * 128].astype(BF)
            .reshape(NKT, 128, 128).transpose(1, 0, 2))
        wv_c = np.ascontiguousarray(
            Wv[:, kv * 128:(kv + 1) * 128].astype(BF)
            .reshape(NKT, 128, 128).transpose(1, 0, 2))
        wo_c = np.ascontiguousarray(
            Wo[h0 * 128:(h0 + 2) * 128, :].astype(BF)
            .reshape(2, 128, D).transpose(1, 0, 2))
        lamn_c = np.array([[-float(lam[h0]), -float(lam[h0 + 1])]], dtype=np.float32)
        in_maps.append({"xt": xt, "wq": wq_c, "wk": wk_c, "wv": wv_c,
                        "wo": wo_c, "lamn": lamn_c})
    return in_maps


def kernel(x, Wq, Wk, Wv, Wo, lam):
    from concourse.bass_utils import run_bass_kernel_spmd

    nc = get_program()
    in_maps = _prep_inputs(np.asarray(x), np.asarray(Wq), np.asarray(Wk),
                           np.asarray(Wv), np.asarray(Wo), np.asarray(lam))
    res = run_bass_kernel_spmd(nc, in_maps, list(range(N_CORES)))
    out = np.zeros((S, D), dtype=np.float32)
    for c in range(N_CORES):
        out += res.results[c]["outp"].astype(np.float32)
    return out.reshape(1, S, D)



# revision 7
# speedup vs baseline: 1.1423x; 1.1423x over previous
"""Trainium2 Bass kernel for nn_ChimeraV2Block (dual-softmax differential
sliding-window attention block, B=1 S=2048 D=2048, 16 q-heads / 4 kv-heads,
head_dim 128, window 512).

Sharding: tensor-parallel over heads across 8 NeuronCores. Core c owns
q-heads {2c, 2c+1} and kv-head c//2 (GQA groups align with the split).
Wq/Wk/Wv column-sharded, Wo row-sharded; the 8 fp32 partial outputs are
summed on the host (the "all-reduce").
"""

import sys

if "/opt/trn_rl_repo" not in sys.path:
    sys.path.insert(0, "/opt/trn_rl_repo")

import numpy as np
import ml_dtypes

BF = ml_dtypes.bfloat16

S = 2048
D = 2048
H = 16
HK = 4
HD = 128
WIN = 512
THETA = 10000.0
N_CORES = 8
NQT = S // 128          # 16 q row-tiles
NKT = D // 128          # 16 contraction tiles for the projections
WMAX = WIN + 128        # 640: max key-window width per q-tile
NEG = -1.0e30

_CACHE = {}


def _tables():
    """RoPE tables [128, S] fp16 with head-dim-duplicated frequencies
    (row p uses invf[p % 64]). The sin table has the rotate-half sign
    folded in and lives at the partition of the SOURCE operand: rows
    64:128 carry -sin (read together with ps[64:128] to produce the low
    output half), rows 0:64 carry +sin. Q tables are pre-scaled by the
    attention scale 1/sqrt(64)."""
    invf = 1.0 / (THETA ** (np.arange(0, HD, 2, dtype=np.float64) / HD))  # [64]
    t = np.arange(S, dtype=np.float64)
    fr = np.outer(invf, t)  # [64, S]
    cosf = np.concatenate([np.cos(fr)] * 2, axis=0)
    sinf = np.concatenate([np.sin(fr), -np.sin(fr)], axis=0)
    return (np.ascontiguousarray(cosf * 0.125, dtype=np.float16),
            np.ascontiguousarray(sinf * 0.125, dtype=np.float16),
            np.ascontiguousarray(cosf, dtype=np.float16),
            np.ascontiguousarray(sinf, dtype=np.float16))


def _masks():
    p = np.arange(128)[:, None]
    c = np.arange(WMAX)[None, :]
    band = (c - p >= 1) & (c - p <= WIN)
    mw = np.where(band, 0.0, NEG).astype(BF)          # [128, 640]
    cc = np.arange(128)[None, :]
    mc = np.where(cc <= p, 0.0, NEG).astype(BF)       # [128, 128] causal
    # edge mask: cols [0,512) allowed, cols [512,640) causal triangle.
    # slicing the last w cols gives the mask for edge q-tiles (qi < 4).
    me = np.zeros((128, WMAX), dtype=BF)
    me[:, WIN:] = mc
    return mw, me


def _build_program():
    import concourse.bacc as bacc
    import concourse.tile as tile
    from concourse import mybir

    bf = mybir.dt.bfloat16
    f32 = mybir.dt.float32
    EXP = mybir.ActivationFunctionType.Exp
    MULT = mybir.AluOpType.mult
    ADD = mybir.AluOpType.add
    MAX = mybir.AluOpType.max
    DIV = mybir.AluOpType.divide

    nc = bacc.Bacc("TRN2", target_bir_lowering=False, debug=False,
                   num_devices=N_CORES)

    xt_d = nc.dram_tensor("xt", [128, NKT, S], bf, kind="ExternalInput")
    wq_d = nc.dram_tensor("wq", [128, NKT, 2, 128], bf, kind="ExternalInput")
    wk_d = nc.dram_tensor("wk", [128, NKT, 128], bf, kind="ExternalInput")
    wv_d = nc.dram_tensor("wv", [128, NKT, 128], bf, kind="ExternalInput")
    wo_d = nc.dram_tensor("wo", [128, 2, D], bf, kind="ExternalInput")
    lamn_d = nc.dram_tensor("lamn", [1, 2], f32, kind="ExternalInput")
    f16 = mybir.dt.float16
    out_d = nc.dram_tensor("outp", [S, D], f16, kind="ExternalOutput")

    tqc_np, tqs_np, tkc_np, tks_np = _tables()
    mw_np, me_np = _masks()
    tqc_d = nc.inline_tensor(tqc_np, "tab_qc")
    tqs_d = nc.inline_tensor(tqs_np, "tab_qs")
    tkc_d = nc.inline_tensor(tkc_np, "tab_kc")
    tks_d = nc.inline_tensor(tks_np, "tab_ks")
    mw_d = nc.inline_tensor(mw_np, "mask_win")
    me_d = nc.inline_tensor(me_np, "mask_edge")
    idb_d = nc.inline_tensor(np.eye(128, dtype=BF), "ident_bf")
    idf_d = nc.inline_tensor(np.eye(128, dtype=np.float32), "ident_f32")

    with tile.TileContext(nc) as tc:
        with tc.tile_pool(name="xpool", bufs=1) as xp, \
             tc.tile_pool(name="wpool", bufs=1) as wp, \
             tc.tile_pool(name="pers", bufs=1) as pers:

            # DMA issue order matters: projection weights + first x chunk
            # first so matmuls start early; bulk x + tables stream in under
            # compute; phase-2/3 constants (masks, wo) last.
            wq = wp.tile([128, NKT, 2, 128], bf)
            for i in range(4):
                nc.sync.dma_start(out=wq[:, 4 * i:4 * i + 4], in_=wq_d[:, 4 * i:4 * i + 4])
            wk = wp.tile([128, NKT, 128], bf)
            nc.sync.dma_start(out=wk[:, 0:8], in_=wk_d[:, 0:8])
            nc.sync.dma_start(out=wk[:, 8:16], in_=wk_d[:, 8:16])
            wv = wp.tile([128, NKT, 128], bf)
            nc.sync.dma_start(out=wv[:, 0:8], in_=wv_d[:, 0:8])
            nc.sync.dma_start(out=wv[:, 8:16], in_=wv_d[:, 8:16])
            idb = wp.tile([128, 128], bf)
            nc.sync.dma_start(out=idb[:], in_=idb_d[:])
            lamn = wp.tile([1, 2], f32)
            nc.sync.dma_start(out=lamn[:], in_=lamn_d[:])

            xts = []
            for nch in range(4):
                xc = xp.tile([128, NKT, 512], bf, tag=f"xt{nch}")
                xts.append(xc)
            tqc = wp.tile([128, S], f16)
            tqs = wp.tile([128, S], f16)
            tkc = wp.tile([128, S], f16)
            tks = wp.tile([128, S], f16)

            def load_chunk(nch):
                sl = slice(nch * 512, (nch + 1) * 512)
                for kti in range(NKT):
                    nc.sync.dma_start(out=xts[nch][:, kti, :], in_=xt_d[:, kti, sl])

            load_chunk(0)
            for i in range(4):
                sl = slice(512 * i, 512 * (i + 1))
                nc.sync.dma_start(out=tqc[:, sl], in_=tqc_d[:, sl])
                nc.sync.dma_start(out=tqs[:, sl], in_=tqs_d[:, sl])
                nc.sync.dma_start(out=tkc[:, sl], in_=tkc_d[:, sl])
                nc.sync.dma_start(out=tks[:, sl], in_=tks_d[:, sl])
            load_chunk(1)
            load_chunk(2)
            load_chunk(3)
            mw = wp.tile([128, WMAX], bf)
            nc.sync.dma_start(out=mw[:], in_=mw_d[:])
            me = wp.tile([128, WMAX], bf)
            nc.sync.dma_start(out=me[:], in_=me_d[:])
            wo = wp.tile([128, 2, D], bf)
            for i in range(4):
                nc.sync.dma_start(out=wo[:, :, 512 * i:512 * (i + 1)],
                                  in_=wo_d[:, :, 512 * i:512 * (i + 1)])
            lamb = wp.tile([128, 2], f32)
            nc.gpsimd.partition_broadcast(lamb[:], lamn[:])

            qt = pers.tile([128, 2, S], bf)      # RoPE'd scaled q, hd-major
            kt = pers.tile([128, S], bf)         # RoPE'd k, hd-major
            vsm = pers.tile([128, NQT, 128], bf)  # v, S-major [s, hd]
            att = pers.tile([128, 2, S], bf)     # attention out^T, hd-major

            # ---- Phase 1: projections + RoPE + v transpose ----
            with tc.tile_pool(name="pp", bufs=1, space="PSUM") as pp, \
                 tc.tile_pool(name="pt", bufs=2) as pt:
                for nch in range(4):
                    sl = slice(nch * 512, (nch + 1) * 512)
                    ps_q0 = pp.tile([128, 512], f32, tag="pq0", bufs=2)
                    ps_q1 = pp.tile([128, 512], f32, tag="pq1", bufs=2)
                    ps_k = pp.tile([128, 512], f32, tag="pk", bufs=1)
                    ps_v = pp.tile([128, 512], f32, tag="pv", bufs=1)
                    for kti in range(NKT):
                        st = kti == 0
                        sp = kti == NKT - 1
                        rhs = xts[nch][:, kti, :]
                        nc.tensor.matmul(ps_q0[:], wq[:, kti, 0, :], rhs, start=st, stop=sp)
                        nc.tensor.matmul(ps_q1[:], wq[:, kti, 1, :], rhs, start=st, stop=sp)
                        nc.tensor.matmul(ps_k[:], wk[:, kti, :], rhs, start=st, stop=sp)
                        nc.tensor.matmul(ps_v[:], wv[:, kti, :], rhs, start=st, stop=sp)
                    for ps, outt, tabc, tabs in (
                            (ps_q0, qt[:, 0, sl], tqc, tqs),
                            (ps_q1, qt[:, 1, sl], tqc, tqs),
                            (ps_k, kt[:, sl], tkc, tks)):
                        # out = ps*cos + rot_half(ps)*sin, sign folded in tabs
                        m1 = pt.tile([128, 512], f32, tag="m1")
                        m2 = pt.tile([128, 512], f32, tag="m2")
                        nc.vector.tensor_mul(m1[:], ps[:], tabc[:, sl])
                        nc.vector.tensor_mul(m2[0:64, :], ps[64:128, :], tabs[64:128, sl])
                        nc.vector.tensor_mul(m2[64:128, :], ps[0:64, :], tabs[0:64, sl])
                        nc.vector.tensor_add(outt[:], m1[:], m2[:])
                    vtmp = pt.tile([128, 512], bf, tag="vtmp")
                    nc.vector.tensor_copy(out=vtmp[:], in_=ps_v[:])
                    ps_tv = pp.tile([128, 4, 128], bf, tag="ptv", bufs=2)
                    for j in range(4):
                        nc.tensor.transpose(ps_tv[:, j, :], vtmp[:, 128 * j:128 * (j + 1)], idb[:])
                    nc.vector.tensor_copy(out=vsm[:, 4 * nch:4 * (nch + 1), :], in_=ps_tv[:])

            # ---- Phase 2: attention ----
            with tc.tile_pool(name="psc", bufs=1, space="PSUM") as psc, \
                 tc.tile_pool(name="pse", bufs=1) as pse, \
                 tc.tile_pool(name="psm", bufs=1) as psm:
                for qi in range(NQT):
                    qsl = slice(qi * 128, (qi + 1) * 128)
                    kw = min(qi + 1, 5)
                    w = kw * 128
                    kstart = max(0, qi - 4)
                    kwin = slice(kstart * 128, kstart * 128 + w)

                    ps_av0 = psc.tile([128, 128], f32, tag="av0", bufs=1)
                    ps_av1 = psc.tile([128, 128], f32, tag="av1", bufs=1)
                    ps_av = [ps_av0, ps_av1]

                    for h in range(2):
                        ps_s1 = psc.tile([128, WMAX], f32, tag="s", bufs=2)
                        ps_s2 = psc.tile([128, WMAX], f32, tag="s", bufs=2)
                        for half, ps in ((0, ps_s1), (1, ps_s2)):
                            hp = slice(64 * half, 64 * half + 64)
                            lhs = qt[hp, h, qsl]
                            msk = mw if qi >= 4 else me[:, WMAX - w:WMAX]
                            wa = min(w, 512)
                            nc.tensor.matmul(ps[:, 0:wa], idb[:], msk[:, 0:wa],
                                             start=True, stop=False)
                            nc.tensor.matmul(ps[:, 0:wa], lhs,
                                             kt[hp, kwin][:, 0:wa],
                                             start=False, stop=True)
                            if w > 512:
                                nc.tensor.matmul(ps[:, 512:w], idb[:],
                                                 msk[:, 512:w],
                                                 start=True, stop=False)
                                nc.tensor.matmul(ps[:, 512:w], lhs,
                                                 kt[hp, kwin][:, 512:w],
                                                 start=False, stop=True)

                        e1 = pse.tile([128, WMAX], bf, tag="e1", bufs=2)
                        e2 = pse.tile([128, WMAX], bf, tag="e2", bufs=2)
                        s1 = psm.tile([128, 1], f32, tag="s1", bufs=4)
                        s2 = psm.tile([128, 1], f32, tag="s2", bufs=4)
                        nc.scalar.activation(out=e1[:, 0:w], in_=ps_s1[:, 0:w],
                                             func=EXP, accum_out=s1[:])
                        nc.scalar.activation(out=e2[:, 0:w], in_=ps_s2[:, 0:w],
                                             func=EXP, accum_out=s2[:])

                        # cneg = -(lam * s1 / s2)   (lamn holds -lam)
                        r2 = psm.tile([128, 1], f32, tag="r2", bufs=4)
                        nc.vector.reciprocal(out=r2[:], in_=s2[:])
                        cneg = psm.tile([128, 1], f32, tag="cneg", bufs=4)
                        nc.vector.scalar_tensor_tensor(
                            out=cneg[:], in0=s1[:], scalar=lamb[:, h:h + 1],
                            in1=r2[:], op0=MULT, op1=MULT)
                        # g0 = e1 + cneg*e2 ; g = relu(g0), accum D'
                        g0 = pse.tile([128, WMAX], bf, tag="g0", bufs=2)
                        g = pse.tile([128, WMAX], bf, tag="g", bufs=2)
                        dsum = psm.tile([128, 1], f32, tag="dsum", bufs=4)
                        nc.vector.scalar_tensor_tensor(
                            out=g0[:, 0:w], in0=e2[:, 0:w], scalar=cneg[:],
                            in1=e1[:, 0:w], op0=MULT, op1=ADD)
                        nc.vector.tensor_scalar(
                            out=g[:, 0:w], in0=g0[:, 0:w], scalar1=0.0,
                            scalar2=0.0, op0=MAX, op1=ADD, accum_out=dsum[:])
                        # recd = 1 / (D' + 1e-6 * s1); gn = g * recd
                        dtmp = psm.tile([128, 1], f32, tag="dtmp", bufs=4)
                        nc.vector.scalar_tensor_tensor(
                            out=dtmp[:], in0=s1[:], scalar=1e-6, in1=dsum[:],
                            op0=MULT, op1=ADD)
                        recd = psm.tile([128, 1], f32, tag="recd", bufs=4)
                        nc.vector.reciprocal(out=recd[:], in_=dtmp[:])
                        gn = pse.tile([128, WMAX], bf, tag="gn", bufs=2)
                        nc.vector.tensor_scalar(
                            out=gn[:, 0:w], in0=g[:, 0:w], scalar1=recd[:],
                            scalar2=0.0, op0=MULT, op1=ADD)

                        # transpose gn -> gT (PSUM) -> SBUF
                        ps_tr = psc.tile([128, kw, 128], bf, tag="trg", bufs=2)
                        for j in range(kw):
                            nc.tensor.transpose(ps_tr[:, j, :],
                                                gn[:, 128 * j:128 * (j + 1)], idb[:])
                        gts = pse.tile([128, 5, 128], bf, tag="gts", bufs=2)
                        nc.vector.tensor_copy(out=gts[:, 0:kw, :], in_=ps_tr[:])

                        # AV: out^T[hd, q] += v[k,:].T-contract over window
                        for j in range(kw):
                            nc.tensor.matmul(ps_av[h][:], vsm[:, kstart + j, :],
                                             gts[:, j, :],
                                             start=(j == 0), stop=(j == kw - 1))

                    nc.vector.tensor_copy(out=att[:, 0, qsl], in_=ps_av[0][:])
                    nc.vector.tensor_copy(out=att[:, 1, qsl], in_=ps_av[1][:])

            # ---- Phase 3: output projection (row-sharded Wo partial) ----
            with tc.tile_pool(name="po", bufs=1, space="PSUM") as po, \
                 tc.tile_pool(name="pos", bufs=1) as pos:
                for qi in range(NQT):
                    qsl = slice(qi * 128, (qi + 1) * 128)
                    for dch in range(4):
                        dsl = slice(dch * 512, (dch + 1) * 512)
                        ps_o = po.tile([128, 512], f32, tag="o", bufs=5)
                        nc.tensor.matmul(ps_o[:], att[:, 0, qsl], wo[:, 0, dsl],
                                         start=True, stop=False)
                        nc.tensor.matmul(ps_o[:], att[:, 1, qsl], wo[:, 1, dsl],
                                         start=False, stop=True)
                        so = pos.tile([128, 512], f16, tag="so", bufs=6)
                        if dch % 2 == 0:
                            nc.vector.tensor_copy(out=so[:], in_=ps_o[:])
                        else:
                            nc.scalar.copy(out=so[:], in_=ps_o[:])
                        nc.sync.dma_start(out=out_d[qsl, dsl], in_=so[:])

    nc.compile()
    return nc


def get_program():
    if "nc" not in _CACHE:
        _CACHE["nc"] = _build_program()
    return _CACHE["nc"]


def _prep_inputs(x, Wq, Wk, Wv, Wo, lam):
    xt = np.ascontiguousarray(x.reshape(S, D).T.astype(BF)
                              .reshape(NKT, 128, S).transpose(1, 0, 2))
    in_maps = []
    for c in range(N_CORES):
        h0 = 2 * c
        kv = c // 2
        wq_c = np.ascontiguousarray(
            Wq[:, h0 * 128:(h0 + 2) * 128].astype(BF)
            .reshape(NKT, 128, 2, 128).transpose(1, 0, 2, 3))
        wk_c = np.ascontiguousarray(
            Wk[:, kv * 128:(kv + 1) * 128].astype(BF)
            .reshape(NKT, 128, 128).transpose(1, 0, 2))
        wv_c = np.ascontiguousarray(
            Wv[:, kv * 128:(kv + 1) * 128].astype(BF)
            .reshape(NKT, 128, 128).transpose(1, 0, 2))
        wo_c = np.ascontiguousarray(
            Wo[h0 * 128:(h0 + 2) * 128, :].astype(BF)
            .reshape(2, 128, D).transpose(1, 0, 2))
        lamn_c = np.array([[-float(lam[h0]), -float(lam[h0 + 1])]], dtype=np.float32)
        in_maps.append({"xt": xt, "wq": wq_c, "wk": wk_c, "wv": wv_c,
                        "wo": wo_c, "lamn": lamn_c})
    return in_maps


def kernel(x, Wq, Wk, Wv, Wo, lam):
    from concourse.bass_utils import run_bass_kernel_spmd

    nc = get_program()
    in_maps = _prep_inputs(np.asarray(x), np.asarray(Wq), np.asarray(Wk),
                           np.asarray(Wv), np.asarray(Wo), np.asarray(lam))
    res = run_bass_kernel_spmd(nc, in_maps, list(range(N_CORES)))
    out = np.zeros((S, D), dtype=np.float32)
    for c in range(N_CORES):
        out += res.results[c]["outp"].astype(np.float32)
    return out.reshape(1, S, D)



# revision 21
# speedup vs baseline: 1.2201x; 1.0681x over previous
"""Trainium2 Bass kernel for nn_ChimeraV2Block (dual-softmax differential
sliding-window attention block, B=1 S=2048 D=2048, 16 q-heads / 4 kv-heads,
head_dim 128, window 512).

Sharding: tensor-parallel over heads across 8 NeuronCores. Core c owns
q-heads {2c, 2c+1} and kv-head c//2 (GQA groups align with the split).
Wq/Wk/Wv column-sharded, Wo row-sharded; the 8 fp32 partial outputs are
summed on the host (the "all-reduce").
"""

import sys

if "/opt/trn_rl_repo" not in sys.path:
    sys.path.insert(0, "/opt/trn_rl_repo")

import numpy as np
import ml_dtypes

BF = ml_dtypes.bfloat16

S = 2048
D = 2048
H = 16
HK = 4
HD = 128
WIN = 512
THETA = 10000.0
N_CORES = 8
NQT = S // 128          # 16 q row-tiles
NKT = D // 128          # 16 contraction tiles for the projections
WMAX = WIN + 128        # 640: max key-window width per q-tile
NEG = -1.0e30

_CACHE = {}


def _tables():
    """RoPE tables [128, S] fp16 with head-dim-duplicated frequencies
    (row p uses invf[p % 64]). The sin table has the rotate-half sign
    folded in and lives at the partition of the SOURCE operand: rows
    64:128 carry -sin (read together with ps[64:128] to produce the low
    output half), rows 0:64 carry +sin. Q tables are pre-scaled by the
    attention scale 1/sqrt(64)."""
    invf = 1.0 / (THETA ** (np.arange(0, HD, 2, dtype=np.float64) / HD))  # [64]
    t = np.arange(S, dtype=np.float64)
    fr = np.outer(invf, t)  # [64, S]
    cosf = np.concatenate([np.cos(fr)] * 2, axis=0)
    sinf = np.concatenate([np.sin(fr), -np.sin(fr)], axis=0)
    return (np.ascontiguousarray(cosf * 0.125, dtype=np.float16),
            np.ascontiguousarray(sinf * 0.125, dtype=np.float16),
            np.ascontiguousarray(cosf, dtype=np.float16),
            np.ascontiguousarray(sinf, dtype=np.float16))


def _masks():
    p = np.arange(128)[:, None]
    c = np.arange(WMAX)[None, :]
    band = (c - p >= 1) & (c - p <= WIN)
    mw = np.where(band, 0.0, NEG).astype(BF)          # [128, 640]
    cc = np.arange(128)[None, :]
    mc = np.where(cc <= p, 0.0, NEG).astype(BF)       # [128, 128] causal
    # edge mask: cols [0,512) allowed, cols [512,640) causal triangle.
    # slicing the last w cols gives the mask for edge q-tiles (qi < 4).
    me = np.zeros((128, WMAX), dtype=BF)
    me[:, WIN:] = mc
    return mw, me


def _build_program():
    import concourse.bacc as bacc
    import concourse.tile as tile
    from concourse import mybir

    bf = mybir.dt.bfloat16
    f32 = mybir.dt.float32
    EXP = mybir.ActivationFunctionType.Exp
    MULT = mybir.AluOpType.mult
    ADD = mybir.AluOpType.add
    MAX = mybir.AluOpType.max
    DIV = mybir.AluOpType.divide

    nc = bacc.Bacc("TRN2", target_bir_lowering=False, debug=False,
                   num_devices=N_CORES)

    xt_d = nc.dram_tensor("xt", [128, NKT, S], bf, kind="ExternalInput")
    wq_d = nc.dram_tensor("wq", [128, NKT, 2, 128], bf, kind="ExternalInput")
    wk_d = nc.dram_tensor("wk", [128, NKT, 128], bf, kind="ExternalInput")
    wv_d = nc.dram_tensor("wv", [128, NKT, 128], bf, kind="ExternalInput")
    wo_d = nc.dram_tensor("wo", [128, 2, D], bf, kind="ExternalInput")
    lamn_d = nc.dram_tensor("lamn", [1, 2], f32, kind="ExternalInput")
    f16 = mybir.dt.float16
    out_d = nc.dram_tensor("outp", [S, D], f16, kind="ExternalOutput")

    tqc_np, tqs_np, tkc_np, tks_np = _tables()
    mw_np, me_np = _masks()
    tqc_d = nc.inline_tensor(tqc_np, "tab_qc")
    tqs_d = nc.inline_tensor(tqs_np, "tab_qs")
    tkc_d = nc.inline_tensor(tkc_np, "tab_kc")
    tks_d = nc.inline_tensor(tks_np, "tab_ks")
    mw_d = nc.inline_tensor(mw_np, "mask_win")
    me_d = nc.inline_tensor(me_np, "mask_edge")
    idb_d = nc.inline_tensor(np.eye(128, dtype=BF), "ident_bf")
    idf_d = nc.inline_tensor(np.eye(128, dtype=np.float32), "ident_f32")

    with tile.TileContext(nc) as tc:
        with tc.tile_pool(name="xpool", bufs=1) as xp, \
             tc.tile_pool(name="wpool", bufs=1) as wp, \
             tc.tile_pool(name="pers", bufs=1) as pers:

            # DMA issue order matters: projection weights + first x chunk
            # first so matmuls start early; bulk x + tables stream in under
            # compute; phase-2/3 constants (masks, wo) last.
            wq = wp.tile([128, NKT, 2, 128], bf)
            for i in range(4):
                nc.sync.dma_start(out=wq[:, 4 * i:4 * i + 4], in_=wq_d[:, 4 * i:4 * i + 4])
            wk = wp.tile([128, NKT, 128], bf)
            nc.sync.dma_start(out=wk[:, 0:8], in_=wk_d[:, 0:8])
            nc.sync.dma_start(out=wk[:, 8:16], in_=wk_d[:, 8:16])
            wv = wp.tile([128, NKT, 128], bf)
            nc.sync.dma_start(out=wv[:, 0:8], in_=wv_d[:, 0:8])
            nc.sync.dma_start(out=wv[:, 8:16], in_=wv_d[:, 8:16])
            idb = wp.tile([128, 128], bf)
            nc.sync.dma_start(out=idb[:], in_=idb_d[:])
            lamn = wp.tile([1, 2], f32)
            nc.sync.dma_start(out=lamn[:], in_=lamn_d[:])

            xts = []
            for nch in range(4):
                xc = xp.tile([128, NKT, 512], bf, tag=f"xt{nch}")
                xts.append(xc)
            tqc = wp.tile([128, S], f16)
            tqs = wp.tile([128, S], f16)
            tkc = wp.tile([128, S], f16)
            tks = wp.tile([128, S], f16)

            def load_chunk(nch, ndma):
                sl = slice(nch * 512, (nch + 1) * 512)
                kstep = NKT // ndma
                for i in range(ndma):
                    ksl = slice(i * kstep, (i + 1) * kstep)
                    nc.sync.dma_start(out=xts[nch][:, ksl, :],
                                      in_=xt_d[:, ksl, sl])

            load_chunk(0, 16)
            for i in range(2):
                sl = slice(1024 * i, 1024 * (i + 1))
                nc.sync.dma_start(out=tqc[:, sl], in_=tqc_d[:, sl])
                nc.sync.dma_start(out=tqs[:, sl], in_=tqs_d[:, sl])
                nc.sync.dma_start(out=tkc[:, sl], in_=tkc_d[:, sl])
                nc.sync.dma_start(out=tks[:, sl], in_=tks_d[:, sl])
            load_chunk(1, 8)
            load_chunk(2, 4)
            load_chunk(3, 4)
            mw = wp.tile([128, WMAX], bf)
            nc.sync.dma_start(out=mw[:], in_=mw_d[:])
            me = wp.tile([128, WMAX], bf)
            nc.sync.dma_start(out=me[:], in_=me_d[:])
            wo = wp.tile([128, 2, D], bf)
            for i in range(4):
                nc.sync.dma_start(out=wo[:, :, 512 * i:512 * (i + 1)],
                                  in_=wo_d[:, :, 512 * i:512 * (i + 1)])
            lamb = wp.tile([128, 2], f32)
            nc.gpsimd.partition_broadcast(lamb[:], lamn[:])

            # q stored zero-padded to full 128 contraction rows per half:
            # qtp0 rows 0:64 hold half-0 q, rows 64:128 are zero; qtp1 is
            # the mirror. A 128-contraction matmul runs at 2x the column
            # rate of a 64-contraction one, so the padded zeros are free.
            qtp0 = pers.tile([128, 2, S], bf)
            qtp1 = pers.tile([128, 2, S], bf)
            kt = pers.tile([128, S], bf)         # RoPE'd k, hd-major
            vsm = pers.tile([128, NQT, 128], bf)  # v, S-major [s, hd]
            att = pers.tile([128, 2, S], bf)     # attention out^T, hd-major
            zeros = pers.tile([128, WMAX], bf)
            nc.gpsimd.memset(qtp0[64:128, :, :], 0.0)
            nc.gpsimd.memset(qtp1[0:64, :, :], 0.0)
            nc.gpsimd.memset(zeros[:], 0.0)

            # ---- Phase 1: projections + RoPE + v transpose ----
            with tc.tile_pool(name="pp", bufs=1, space="PSUM") as pp, \
                 tc.tile_pool(name="pt", bufs=2) as pt:
                for nch in range(4):
                    sl = slice(nch * 512, (nch + 1) * 512)
                    ps_q0 = pp.tile([128, 512], f32, tag="pq0", bufs=2)
                    ps_q1 = pp.tile([128, 512], f32, tag="pq1", bufs=2)
                    ps_k = pp.tile([128, 512], f32, tag="pk", bufs=1)
                    ps_v = pp.tile([128, 512], f32, tag="pv", bufs=1)
                    for kti in range(NKT):
                        st = kti == 0
                        sp = kti == NKT - 1
                        rhs = xts[nch][:, kti, :]
                        nc.tensor.matmul(ps_q0[:], wq[:, kti, 0, :], rhs, start=st, stop=sp)
                        nc.tensor.matmul(ps_q1[:], wq[:, kti, 1, :], rhs, start=st, stop=sp)
                        nc.tensor.matmul(ps_k[:], wk[:, kti, :], rhs, start=st, stop=sp)
                        nc.tensor.matmul(ps_v[:], wv[:, kti, :], rhs, start=st, stop=sp)
                    for ps, outlo, outhi, tabc, tabs in (
                            (ps_q0, qtp0[0:64, 0, sl], qtp1[64:128, 0, sl], tqc, tqs),
                            (ps_q1, qtp0[0:64, 1, sl], qtp1[64:128, 1, sl], tqc, tqs),
                            (ps_k, None, None, tkc, tks)):
                        # out = ps*cos + rot_half(ps)*sin, sign folded in tabs
                        m1 = pt.tile([128, 512], f32, tag="m1")
                        m2 = pt.tile([128, 512], f32, tag="m2")
                        nc.vector.tensor_mul(m1[:], ps[:], tabc[:, sl])
                        nc.vector.tensor_mul(m2[0:64, :], ps[64:128, :], tabs[64:128, sl])
                        nc.vector.tensor_mul(m2[64:128, :], ps[0:64, :], tabs[0:64, sl])
                        if outlo is None:
                            nc.vector.tensor_add(kt[:, sl], m1[:], m2[:])
                        else:
                            nc.vector.tensor_add(outlo, m1[0:64, :], m2[0:64, :])
                            nc.vector.tensor_add(outhi, m1[64:128, :], m2[64:128, :])
                    vtmp = pt.tile([128, 512], bf, tag="vtmp")
                    nc.vector.tensor_copy(out=vtmp[:], in_=ps_v[:])
                    ps_tv = pp.tile([128, 4, 128], bf, tag="ptv", bufs=2)
                    for j in range(4):
                        nc.tensor.transpose(ps_tv[:, j, :], vtmp[:, 128 * j:128 * (j + 1)], idb[:])
                    nc.vector.tensor_copy(out=vsm[:, 4 * nch:4 * (nch + 1), :], in_=ps_tv[:])

            # ---- Phase 2: attention ----
            with tc.tile_pool(name="psc", bufs=1, space="PSUM") as psc, \
                 tc.tile_pool(name="pse", bufs=1) as pse, \
                 tc.tile_pool(name="psm", bufs=1) as psm:
                for qi in range(NQT):
                    qsl = slice(qi * 128, (qi + 1) * 128)
                    kw = min(qi + 1, 5)
                    w = kw * 128
                    kstart = max(0, qi - 4)
                    kwin = slice(kstart * 128, kstart * 128 + w)

                    # both heads' AV share one 256-wide PSUM tile
                    ps_av = psc.tile([128, 2, 128], f32, tag="av", bufs=2)
                    gts = pse.tile([128, 2, 5, 128], bf, tag="gts", bufs=2)

                    for h in range(2):
                        ps_s1 = psc.tile([128, WMAX], f32, tag="s", bufs=2)
                        ps_s2 = psc.tile([128, WMAX], f32, tag="s", bufs=2)
                        for ps, lhsq in ((ps_s1, qtp0), (ps_s2, qtp1)):
                            lhs = lhsq[:, h, qsl]
                            if qi >= 4:
                                # mask matmul covers the causal edge in block
                                # 0 (cols 128:512 of mw are all zero)
                                nc.tensor.matmul(ps[:, 0:512], idb[:], mw[:, 0:512],
                                                 start=True, stop=False)
                                nc.tensor.matmul(ps[:, 0:512], lhs,
                                                 kt[:, kwin][:, 0:512],
                                                 start=False, stop=True)
                                nc.tensor.matmul(ps[:, 512:640], idb[:],
                                                 mw[:, 512:640],
                                                 start=True, stop=False)
                                nc.tensor.matmul(ps[:, 512:640], lhs,
                                                 kt[:, kwin][:, 512:640],
                                                 start=False, stop=True)
                            else:
                                nc.tensor.matmul(ps[:, 0:w], idb[:],
                                                 me[:, WMAX - w:WMAX],
                                                 start=True, stop=False)
                                nc.tensor.matmul(ps[:, 0:w], lhs,
                                                 kt[:, kwin][:, 0:w],
                                                 start=False, stop=True)

                        e1 = pse.tile([128, WMAX], bf, tag="e1", bufs=2)
                        e2 = pse.tile([128, WMAX], bf, tag="e2", bufs=2)
                        s1 = psm.tile([128, 1], f32, tag="s1", bufs=4)
                        s2 = psm.tile([128, 1], f32, tag="s2", bufs=4)
                        nc.scalar.activation(out=e1[:, 0:w], in_=ps_s1[:, 0:w],
                                             func=EXP, accum_out=s1[:])
                        nc.scalar.activation(out=e2[:, 0:w], in_=ps_s2[:, 0:w],
                                             func=EXP, accum_out=s2[:])

                        # cneg = -(lam * s1 / s2)   (lamn holds -lam)
                        r2 = psm.tile([128, 1], f32, tag="r2", bufs=4)
                        nc.vector.reciprocal(out=r2[:], in_=s2[:])
                        cneg = psm.tile([128, 1], f32, tag="cneg", bufs=4)
                        nc.vector.scalar_tensor_tensor(
                            out=cneg[:], in0=s1[:], scalar=lamb[:, h:h + 1],
                            in1=r2[:], op0=MULT, op1=MULT)
                        # g0 = e1 + cneg*e2 ; g = relu(g0), accum D'
                        g0 = pse.tile([128, WMAX], bf, tag="g0", bufs=2)
                        g = pse.tile([128, WMAX], bf, tag="g", bufs=2)
                        dsum = psm.tile([128, 1], f32, tag="dsum", bufs=4)
                        nc.vector.scalar_tensor_tensor(
                            out=g0[:, 0:w], in0=e2[:, 0:w], scalar=cneg[:],
                            in1=e1[:, 0:w], op0=MULT, op1=ADD)
                        nc.vector.tensor_scalar(
                            out=g[:, 0:w], in0=g0[:, 0:w], scalar1=0.0,
                            scalar2=0.0, op0=MAX, op1=ADD, accum_out=dsum[:])
                        # recd = 1 / (D' + 1e-6 * s1); gn = g * recd
                        dtmp = psm.tile([128, 1], f32, tag="dtmp", bufs=4)
                        nc.vector.scalar_tensor_tensor(
                            out=dtmp[:], in0=s1[:], scalar=1e-6, in1=dsum[:],
                            op0=MULT, op1=ADD)
                        recd = psm.tile([128, 1], f32, tag="recd", bufs=4)
                        nc.vector.reciprocal(out=recd[:], in_=dtmp[:])
                        gn = pse.tile([128, WMAX], bf, tag="gn", bufs=2)
                        nc.vector.tensor_scalar(
                            out=gn[:, 0:w], in0=g[:, 0:w], scalar1=recd[:],
                            scalar2=0.0, op0=MULT, op1=ADD)

                        # transpose gn -> gT (PSUM) -> SBUF
                        ps_tr = psc.tile([128, kw, 128], bf, tag="trg", bufs=2)
                        for j in range(kw):
                            nc.tensor.transpose(ps_tr[:, j, :],
                                                gn[:, 128 * j:128 * (j + 1)], idb[:])
                        if h == 0:
                            nc.vector.tensor_copy(out=gts[:, 0, 0:kw, :], in_=ps_tr[:])
                        else:
                            nc.scalar.copy(out=gts[:, 1, 0:kw, :], in_=ps_tr[:])

                    # AV for both heads at once: [k,hd]^T-contract x [k, 2*128]
                    for j in range(kw):
                        nc.tensor.matmul(ps_av[:], vsm[:, kstart + j, :],
                                         gts[:, :, j, :],
                                         start=(j == 0), stop=(j == kw - 1))

                    nc.vector.tensor_copy(out=att[:, :, qsl], in_=ps_av[:])

            # ---- Phase 3: output projection (row-sharded Wo partial) ----
            with tc.tile_pool(name="po", bufs=1, space="PSUM") as po, \
                 tc.tile_pool(name="pos", bufs=1) as pos:
                for qi in range(NQT):
                    qsl = slice(qi * 128, (qi + 1) * 128)
                    so = pos.tile([128, 2048], f16, tag="so", bufs=2)
                    for dch in range(4):
                        dsl = slice(dch * 512, (dch + 1) * 512)
                        ps_o = po.tile([128, 512], f32, tag="o", bufs=5)
                        nc.tensor.matmul(ps_o[:], att[:, 0, qsl], wo[:, 0, dsl],
                                         start=True, stop=False)
                        nc.tensor.matmul(ps_o[:], att[:, 1, qsl], wo[:, 1, dsl],
                                         start=False, stop=True)
                        if dch % 2 == 0:
                            nc.vector.tensor_copy(out=so[:, dsl], in_=ps_o[:])
                        else:
                            nc.scalar.copy(out=so[:, dsl], in_=ps_o[:])
                        if dch % 2 == 1:
                            dsl2 = slice((dch - 1) * 512, (dch + 1) * 512)
                            nc.sync.dma_start(out=out_d[qsl, dsl2], in_=so[:, dsl2])

    nc.compile()
    return nc


def get_program():
    if "nc" not in _CACHE:
        _CACHE["nc"] = _build_program()
    return _CACHE["nc"]


def _prep_inputs(x, Wq, Wk, Wv, Wo, lam):
    xt = np.ascontiguousarray(x.reshape(S, D).T.astype(BF)
                              .reshape(NKT, 128, S).transpose(1, 0, 2))
    in_maps = []
    for c in range(N_CORES):
        h0 = 2 * c
        kv = c // 2
        wq_c = np.ascontiguousarray(
            Wq[:, h0 * 128:(h0 + 2) * 128].astype(BF)
            .reshape(NKT, 128, 2, 128).transpose(1, 0, 2, 3))
        wk_c = np.ascontiguousarray(
            Wk[:, kv * 128:(kv + 1) * 128].astype(BF)
            .reshape(NKT, 128, 128).transpose(1, 0, 2))
        wv_c = np.ascontiguousarray(
            Wv[:, kv * 128:(kv + 1) * 128].astype(BF)
            .reshape(NKT, 128, 128).transpose(1, 0, 2))
        wo_c = np.ascontiguousarray(
            Wo[h0 * 128:(h0 + 2) * 128, :].astype(BF)
            .reshape(2, 128, D).transpose(1, 0, 2))
        lamn_c = np.array([[-float(lam[h0]), -float(lam[h0 + 1])]], dtype=np.float32)
        in_maps.append({"xt": xt, "wq": wq_c, "wk": wk_c, "wv": wv_c,
                        "wo": wo_c, "lamn": lamn_c})
    return in_maps


def kernel(x, Wq, Wk, Wv, Wo, lam):
    from concourse.bass_utils import run_bass_kernel_spmd

    nc = get_program()
    in_maps = _prep_inputs(np.asarray(x), np.asarray(Wq), np.asarray(Wk),
                           np.asarray(Wv), np.asarray(Wo), np.asarray(lam))
    res = run_bass_kernel_spmd(nc, in_maps, list(range(N_CORES)))
    out = np.zeros((S, D), dtype=np.float32)
    for c in range(N_CORES):
        out += res.results[c]["outp"].astype(np.float32)
    return out.reshape(1, S, D)



# revision 23
# speedup vs baseline: 1.2695x; 1.0405x over previous
"""Trainium2 Bass kernel for nn_ChimeraV2Block (dual-softmax differential
sliding-window attention block, B=1 S=2048 D=2048, 16 q-heads / 4 kv-heads,
head_dim 128, window 512).

Sharding: tensor-parallel over heads across 8 NeuronCores. Core c owns
q-heads {2c, 2c+1} and kv-head c//2 (GQA groups align with the split).
Wq/Wk/Wv column-sharded, Wo row-sharded; the 8 fp32 partial outputs are
summed on the host (the "all-reduce").
"""

import sys

if "/opt/trn_rl_repo" not in sys.path:
    sys.path.insert(0, "/opt/trn_rl_repo")

import numpy as np
import ml_dtypes

BF = ml_dtypes.bfloat16

S = 2048
D = 2048
H = 16
HK = 4
HD = 128
WIN = 512
THETA = 10000.0
N_CORES = 8
NQT = S // 128          # 16 q row-tiles
NKT = D // 128          # 16 contraction tiles for the projections
WMAX = WIN + 128        # 640: max key-window width per q-tile
NEG = -1.0e30

_CACHE = {}


def _tables():
    """RoPE tables [128, S] fp16 with head-dim-duplicated frequencies
    (row p uses invf[p % 64]). The sin table has the rotate-half sign
    folded in and lives at the partition of the SOURCE operand: rows
    64:128 carry -sin (read together with ps[64:128] to produce the low
    output half), rows 0:64 carry +sin. Q tables are pre-scaled by the
    attention scale 1/sqrt(64)."""
    invf = 1.0 / (THETA ** (np.arange(0, HD, 2, dtype=np.float64) / HD))  # [64]
    t = np.arange(S, dtype=np.float64)
    fr = np.outer(invf, t)  # [64, S]
    cosf = np.concatenate([np.cos(fr)] * 2, axis=0)
    sinf = np.concatenate([np.sin(fr), -np.sin(fr)], axis=0)
    return (np.ascontiguousarray(cosf * 0.125, dtype=np.float16),
            np.ascontiguousarray(sinf * 0.125, dtype=np.float16),
            np.ascontiguousarray(cosf, dtype=np.float16),
            np.ascontiguousarray(sinf, dtype=np.float16))


def _masks():
    p = np.arange(128)[:, None]
    c = np.arange(WMAX)[None, :]
    band = (c - p >= 1) & (c - p <= WIN)
    mw = np.where(band, 0.0, NEG).astype(BF)          # [128, 640]
    cc = np.arange(128)[None, :]
    mc = np.where(cc <= p, 0.0, NEG).astype(BF)       # [128, 128] causal
    # edge mask: cols [0,512) allowed, cols [512,640) causal triangle.
    # slicing the last w cols gives the mask for edge q-tiles (qi < 4).
    me = np.zeros((128, WMAX), dtype=BF)
    me[:, WIN:] = mc
    return mw, me


def _build_program():
    import concourse.bacc as bacc
    import concourse.tile as tile
    from concourse import mybir

    bf = mybir.dt.bfloat16
    f32 = mybir.dt.float32
    EXP = mybir.ActivationFunctionType.Exp
    MULT = mybir.AluOpType.mult
    ADD = mybir.AluOpType.add
    MAX = mybir.AluOpType.max
    DIV = mybir.AluOpType.divide

    nc = bacc.Bacc("TRN2", target_bir_lowering=False, debug=False,
                   num_devices=N_CORES)

    xt_d = nc.dram_tensor("xt", [128, NKT, S], bf, kind="ExternalInput")
    wq_d = nc.dram_tensor("wq", [128, NKT, 2, 128], bf, kind="ExternalInput")
    wk_d = nc.dram_tensor("wk", [128, NKT, 128], bf, kind="ExternalInput")
    wv_d = nc.dram_tensor("wv", [128, NKT, 128], bf, kind="ExternalInput")
    wo_d = nc.dram_tensor("wo", [128, 2, D], bf, kind="ExternalInput")
    lamn_d = nc.dram_tensor("lamn", [1, 2], f32, kind="ExternalInput")
    f16 = mybir.dt.float16
    out_d = nc.dram_tensor("outp", [S, D], f16, kind="ExternalOutput")

    tqc_np, tqs_np, tkc_np, tks_np = _tables()
    mw_np, me_np = _masks()
    tqc_d = nc.inline_tensor(tqc_np, "tab_qc")
    tqs_d = nc.inline_tensor(tqs_np, "tab_qs")
    tkc_d = nc.inline_tensor(tkc_np, "tab_kc")
    tks_d = nc.inline_tensor(tks_np, "tab_ks")
    mw_d = nc.inline_tensor(mw_np, "mask_win")
    me_d = nc.inline_tensor(me_np, "mask_edge")
    idb_d = nc.inline_tensor(np.eye(128, dtype=BF), "ident_bf")
    idf_d = nc.inline_tensor(np.eye(128, dtype=np.float32), "ident_f32")

    with tile.TileContext(nc) as tc:
        with tc.tile_pool(name="xpool", bufs=1) as xp, \
             tc.tile_pool(name="wpool", bufs=1) as wp, \
             tc.tile_pool(name="pers", bufs=1) as pers:

            # DMA issue order matters: projection weights + first x chunk
            # first so matmuls start early; bulk x + tables stream in under
            # compute; phase-2/3 constants (masks, wo) last.
            wq = wp.tile([128, NKT, 2, 128], bf)
            for i in range(4):
                nc.sync.dma_start(out=wq[:, 4 * i:4 * i + 4], in_=wq_d[:, 4 * i:4 * i + 4])
            wk = wp.tile([128, NKT, 128], bf)
            nc.sync.dma_start(out=wk[:, 0:8], in_=wk_d[:, 0:8])
            nc.sync.dma_start(out=wk[:, 8:16], in_=wk_d[:, 8:16])
            wv = wp.tile([128, NKT, 128], bf)
            nc.sync.dma_start(out=wv[:, 0:8], in_=wv_d[:, 0:8])
            nc.sync.dma_start(out=wv[:, 8:16], in_=wv_d[:, 8:16])
            idb = wp.tile([128, 128], bf)
            nc.sync.dma_start(out=idb[:], in_=idb_d[:])
            lamn = wp.tile([1, 2], f32)
            nc.sync.dma_start(out=lamn[:], in_=lamn_d[:])

            xts = []
            for nch in range(4):
                xc = xp.tile([128, NKT, 512], bf, tag=f"xt{nch}")
                xts.append(xc)
            tqc = wp.tile([128, S], f16)
            tqs = wp.tile([128, S], f16)
            tkc = wp.tile([128, S], f16)
            tks = wp.tile([128, S], f16)

            def load_chunk(nch, ndma):
                sl = slice(nch * 512, (nch + 1) * 512)
                kstep = NKT // ndma
                for i in range(ndma):
                    ksl = slice(i * kstep, (i + 1) * kstep)
                    nc.sync.dma_start(out=xts[nch][:, ksl, :],
                                      in_=xt_d[:, ksl, sl])

            load_chunk(0, 16)
            for i in range(2):
                sl = slice(1024 * i, 1024 * (i + 1))
                nc.sync.dma_start(out=tqc[:, sl], in_=tqc_d[:, sl])
                nc.sync.dma_start(out=tqs[:, sl], in_=tqs_d[:, sl])
                nc.sync.dma_start(out=tkc[:, sl], in_=tkc_d[:, sl])
                nc.sync.dma_start(out=tks[:, sl], in_=tks_d[:, sl])
            load_chunk(1, 8)
            load_chunk(2, 4)
            load_chunk(3, 4)
            mw = wp.tile([128, WMAX], bf)
            nc.sync.dma_start(out=mw[:], in_=mw_d[:])
            me = wp.tile([128, WMAX], bf)
            nc.sync.dma_start(out=me[:], in_=me_d[:])
            wo = wp.tile([128, 2, D], bf)
            for i in range(4):
                nc.sync.dma_start(out=wo[:, :, 512 * i:512 * (i + 1)],
                                  in_=wo_d[:, :, 512 * i:512 * (i + 1)])
            lamb = wp.tile([128, 2], f32)
            nc.gpsimd.partition_broadcast(lamb[:], lamn[:])

            # q stored zero-padded to full 128 contraction rows per half:
            # qtp0 rows 0:64 hold half-0 q, rows 64:128 are zero; qtp1 is
            # the mirror. A 128-contraction matmul runs at 2x the column
            # rate of a 64-contraction one, so the padded zeros are free.
            qtp0 = pers.tile([128, 2, S], bf)
            qtp1 = pers.tile([128, 2, S], bf)
            kt = pers.tile([128, S], bf)         # RoPE'd k, hd-major
            vsm = pers.tile([128, NQT, 128], bf)  # v, S-major [s, hd]
            att = pers.tile([128, 2, S], bf)     # attention out^T, hd-major
            zeros = pers.tile([128, WMAX], bf)
            nc.gpsimd.memset(qtp0[64:128, :, :], 0.0)
            nc.gpsimd.memset(qtp1[0:64, :, :], 0.0)
            nc.gpsimd.memset(zeros[:], 0.0)

            # ---- Phase 1: projections + RoPE + v transpose ----
            with tc.tile_pool(name="pp", bufs=1, space="PSUM") as pp, \
                 tc.tile_pool(name="pt", bufs=2) as pt:
                for nch in range(4):
                    sl = slice(nch * 512, (nch + 1) * 512)
                    ps_q0 = pp.tile([128, 512], f32, tag="pq0", bufs=2)
                    ps_q1 = pp.tile([128, 512], f32, tag="pq1", bufs=2)
                    ps_k = pp.tile([128, 512], f32, tag="pk", bufs=1)
                    ps_v = pp.tile([128, 512], f32, tag="pv", bufs=1)
                    for kti in range(NKT):
                        st = kti == 0
                        sp = kti == NKT - 1
                        rhs = xts[nch][:, kti, :]
                        nc.tensor.matmul(ps_q0[:], wq[:, kti, 0, :], rhs, start=st, stop=sp)
                        nc.tensor.matmul(ps_q1[:], wq[:, kti, 1, :], rhs, start=st, stop=sp)
                        nc.tensor.matmul(ps_k[:], wk[:, kti, :], rhs, start=st, stop=sp)
                        nc.tensor.matmul(ps_v[:], wv[:, kti, :], rhs, start=st, stop=sp)
                    for ps, outlo, outhi, tabc, tabs in (
                            (ps_q0, qtp0[0:64, 0, sl], qtp1[64:128, 0, sl], tqc, tqs),
                            (ps_q1, qtp0[0:64, 1, sl], qtp1[64:128, 1, sl], tqc, tqs),
                            (ps_k, None, None, tkc, tks)):
                        # out = ps*cos + rot_half(ps)*sin, sign folded in tabs
                        m1 = pt.tile([128, 512], f32, tag="m1")
                        m2 = pt.tile([128, 512], f32, tag="m2")
                        nc.vector.tensor_mul(m1[:], ps[:], tabc[:, sl])
                        nc.vector.tensor_mul(m2[0:64, :], ps[64:128, :], tabs[64:128, sl])
                        nc.vector.tensor_mul(m2[64:128, :], ps[0:64, :], tabs[0:64, sl])
                        if outlo is None:
                            nc.vector.tensor_add(kt[:, sl], m1[:], m2[:])
                        else:
                            nc.vector.tensor_add(outlo, m1[0:64, :], m2[0:64, :])
                            nc.vector.tensor_add(outhi, m1[64:128, :], m2[64:128, :])
                    vtmp = pt.tile([128, 512], bf, tag="vtmp")
                    nc.vector.tensor_copy(out=vtmp[:], in_=ps_v[:])
                    ps_tv = pp.tile([128, 4, 128], bf, tag="ptv", bufs=2)
                    for j in range(4):
                        nc.tensor.transpose(ps_tv[:, j, :], vtmp[:, 128 * j:128 * (j + 1)], idb[:])
                    nc.vector.tensor_copy(out=vsm[:, 4 * nch:4 * (nch + 1), :], in_=ps_tv[:])

            # ---- Phase 2: attention ----
            with tc.tile_pool(name="psc", bufs=1, space="PSUM") as psc, \
                 tc.tile_pool(name="pse", bufs=1) as pse, \
                 tc.tile_pool(name="psm", bufs=1) as psm:
                for qi in range(NQT):
                    qsl = slice(qi * 128, (qi + 1) * 128)
                    kw = min(qi + 1, 5)
                    w = kw * 128
                    kstart = max(0, qi - 4)
                    kwin = slice(kstart * 128, kstart * 128 + w)

                    # both heads' AV share one 256-wide PSUM tile
                    ps_av = psc.tile([128, 2, 128], f32, tag="av", bufs=1)
                    gts = pse.tile([128, 2, 5, 128], bf, tag="gts", bufs=2)

                    for h in range(2):
                        ps_s1 = psc.tile([128, WMAX], f32, tag="s", bufs=2)
                        ps_s2 = psc.tile([128, WMAX], f32, tag="s", bufs=2)
                        for ps, lhsq in ((ps_s1, qtp0), (ps_s2, qtp1)):
                            lhs = lhsq[:, h, qsl]
                            if qi >= 4:
                                # mask matmul covers the causal edge in block
                                # 0 (cols 128:512 of mw are all zero)
                                nc.tensor.matmul(ps[:, 0:512], idb[:], mw[:, 0:512],
                                                 start=True, stop=False)
                                nc.tensor.matmul(ps[:, 0:512], lhs,
                                                 kt[:, kwin][:, 0:512],
                                                 start=False, stop=True)
                                nc.tensor.matmul(ps[:, 512:640], idb[:],
                                                 mw[:, 512:640],
                                                 start=True, stop=False)
                                nc.tensor.matmul(ps[:, 512:640], lhs,
                                                 kt[:, kwin][:, 512:640],
                                                 start=False, stop=True)
                            else:
                                nc.tensor.matmul(ps[:, 0:w], idb[:],
                                                 me[:, WMAX - w:WMAX],
                                                 start=True, stop=False)
                                nc.tensor.matmul(ps[:, 0:w], lhs,
                                                 kt[:, kwin][:, 0:w],
                                                 start=False, stop=True)

                        e1 = pse.tile([128, WMAX], bf, tag="e1", bufs=2)
                        e2 = pse.tile([128, WMAX], bf, tag="e2", bufs=2)
                        s1 = psm.tile([128, 1], f32, tag="s1", bufs=4)
                        s2 = psm.tile([128, 1], f32, tag="s2", bufs=4)
                        nc.scalar.activation(out=e1[:, 0:w], in_=ps_s1[:, 0:w],
                                             func=EXP, accum_out=s1[:])
                        nc.scalar.activation(out=e2[:, 0:w], in_=ps_s2[:, 0:w],
                                             func=EXP, accum_out=s2[:])

                        # cneg = -(lam * s1 / s2)   (lamn holds -lam)
                        r2 = psm.tile([128, 1], f32, tag="r2", bufs=4)
                        nc.vector.reciprocal(out=r2[:], in_=s2[:])
                        cneg = psm.tile([128, 1], f32, tag="cneg", bufs=4)
                        nc.vector.scalar_tensor_tensor(
                            out=cneg[:], in0=s1[:], scalar=lamb[:, h:h + 1],
                            in1=r2[:], op0=MULT, op1=MULT)
                        # g0 = e1 + cneg*e2 ; g = relu(g0), accum D'
                        g0 = pse.tile([128, WMAX], bf, tag="g0", bufs=2)
                        g = pse.tile([128, WMAX], bf, tag="g", bufs=2)
                        dsum = psm.tile([128, 1], f32, tag="dsum", bufs=4)
                        nc.vector.scalar_tensor_tensor(
                            out=g0[:, 0:w], in0=e2[:, 0:w], scalar=cneg[:],
                            in1=e1[:, 0:w], op0=MULT, op1=ADD)
                        nc.vector.tensor_scalar(
                            out=g[:, 0:w], in0=g0[:, 0:w], scalar1=0.0,
                            scalar2=0.0, op0=MAX, op1=ADD, accum_out=dsum[:])
                        # recd = 1 / (D' + 1e-6 * s1); gn = g * recd
                        dtmp = psm.tile([128, 1], f32, tag="dtmp", bufs=4)
                        nc.vector.scalar_tensor_tensor(
                            out=dtmp[:], in0=s1[:], scalar=1e-6, in1=dsum[:],
                            op0=MULT, op1=ADD)
                        recd = psm.tile([128, 1], f32, tag="recd", bufs=4)
                        nc.vector.reciprocal(out=recd[:], in_=dtmp[:])
                        gn = pse.tile([128, WMAX], bf, tag="gn", bufs=2)
                        nc.vector.tensor_scalar(
                            out=gn[:, 0:w], in0=g[:, 0:w], scalar1=recd[:],
                            scalar2=0.0, op0=MULT, op1=ADD)

                        # transpose gn -> gT (PSUM) -> SBUF
                        ps_tr = psc.tile([128, kw, 128], bf, tag="trg", bufs=2)
                        for j in range(kw):
                            nc.tensor.transpose(ps_tr[:, j, :],
                                                gn[:, 128 * j:128 * (j + 1)], idb[:])
                        if h == 0:
                            nc.vector.tensor_copy(out=gts[:, 0, 0:kw, :], in_=ps_tr[:])
                        else:
                            nc.scalar.copy(out=gts[:, 1, 0:kw, :], in_=ps_tr[:])

                    # AV for both heads at once: [k,hd]^T-contract x [k, 2*128]
                    for j in range(kw):
                        nc.tensor.matmul(ps_av[:], vsm[:, kstart + j, :],
                                         gts[:, :, j, :],
                                         start=(j == 0), stop=(j == kw - 1))

                    nc.vector.tensor_copy(out=att[:, :, qsl], in_=ps_av[:])

                    # out-projection for this q-tile, interleaved so the PE
                    # fills attention bubbles and the output DMA spreads out
                    so = pse.tile([128, 2048], f16, tag="so", bufs=2)
                    for dch in range(4):
                        dsl = slice(dch * 512, (dch + 1) * 512)
                        ps_o = psc.tile([128, 512], f32, tag="o", bufs=1)
                        nc.tensor.matmul(ps_o[:], att[:, 0, qsl], wo[:, 0, dsl],
                                         start=True, stop=False)
                        nc.tensor.matmul(ps_o[:], att[:, 1, qsl], wo[:, 1, dsl],
                                         start=False, stop=True)
                        if dch % 2 == 0:
                            nc.vector.tensor_copy(out=so[:, dsl], in_=ps_o[:])
                        else:
                            nc.scalar.copy(out=so[:, dsl], in_=ps_o[:])
                        if dch % 2 == 1:
                            dsl2 = slice((dch - 1) * 512, (dch + 1) * 512)
                            nc.sync.dma_start(out=out_d[qsl, dsl2], in_=so[:, dsl2])

    nc.compile()
    return nc


def get_program():
    if "nc" not in _CACHE:
        _CACHE["nc"] = _build_program()
    return _CACHE["nc"]


def _prep_inputs(x, Wq, Wk, Wv, Wo, lam):
    xt = np.ascontiguousarray(x.reshape(S, D).T.astype(BF)
                              .reshape(NKT, 128, S).transpose(1, 0, 2))
    in_maps = []
    for c in range(N_CORES):
        h0 = 2 * c
        kv = c // 2
        wq_c = np.ascontiguousarray(
            Wq[:, h0 * 128:(h0 + 2) * 128].astype(BF)
            .reshape(NKT, 128, 2, 128).transpose(1, 0, 2, 3))
        wk_c = np.ascontiguousarray(
            Wk[:, kv * 128:(kv + 1) * 128].astype(BF)
            .reshape(NKT, 128, 128).transpose(1, 0, 2))
        wv_c = np.ascontiguousarray(
            Wv[:, kv * 128:(kv + 1) * 128].astype(BF)
            .reshape(NKT, 128, 128).transpose(1, 0, 2))
        wo_c = np.ascontiguousarray(
            Wo[h0 * 128:(h0 + 2) * 128, :].astype(BF)
            .reshape(2, 128, D).transpose(1, 0, 2))
        lamn_c = np.array([[-float(lam[h0]), -float(lam[h0 + 1])]], dtype=np.float32)
        in_maps.append({"xt": xt, "wq": wq_c, "wk": wk_c, "wv": wv_c,
                        "wo": wo_c, "lamn": lamn_c})
    return in_maps


def kernel(x, Wq, Wk, Wv, Wo, lam):
    from concourse.bass_utils import run_bass_kernel_spmd

    nc = get_program()
    in_maps = _prep_inputs(np.asarray(x), np.asarray(Wq), np.asarray(Wk),
                           np.asarray(Wv), np.asarray(Wo), np.asarray(lam))
    res = run_bass_kernel_spmd(nc, in_maps, list(range(N_CORES)))
    out = np.zeros((S, D), dtype=np.float32)
    for c in range(N_CORES):
        out += res.results[c]["outp"].astype(np.float32)
    return out.reshape(1, S, D)



# revision 24
# speedup vs baseline: 1.3327x; 1.0498x over previous
"""Trainium2 Bass kernel for nn_ChimeraV2Block (dual-softmax differential
sliding-window attention block, B=1 S=2048 D=2048, 16 q-heads / 4 kv-heads,
head_dim 128, window 512).

Sharding: tensor-parallel over heads across 8 NeuronCores. Core c owns
q-heads {2c, 2c+1} and kv-head c//2 (GQA groups align with the split).
Wq/Wk/Wv column-sharded, Wo row-sharded; the 8 fp32 partial outputs are
summed on the host (the "all-reduce").
"""

import sys

if "/opt/trn_rl_repo" not in sys.path:
    sys.path.insert(0, "/opt/trn_rl_repo")

import numpy as np
import ml_dtypes

BF = ml_dtypes.bfloat16

S = 2048
D = 2048
H = 16
HK = 4
HD = 128
WIN = 512
THETA = 10000.0
N_CORES = 8
NQT = S // 128          # 16 q row-tiles
NKT = D // 128          # 16 contraction tiles for the projections
WMAX = WIN + 128        # 640: max key-window width per q-tile
NEG = -1.0e30

_CACHE = {}


def _tables():
    """RoPE tables [128, S] fp16 with head-dim-duplicated frequencies
    (row p uses invf[p % 64]). The sin table has the rotate-half sign
    folded in and lives at the partition of the SOURCE operand: rows
    64:128 carry -sin (read together with ps[64:128] to produce the low
    output half), rows 0:64 carry +sin. Q tables are pre-scaled by the
    attention scale 1/sqrt(64)."""
    invf = 1.0 / (THETA ** (np.arange(0, HD, 2, dtype=np.float64) / HD))  # [64]
    t = np.arange(S, dtype=np.float64)
    fr = np.outer(invf, t)  # [64, S]
    cosf = np.concatenate([np.cos(fr)] * 2, axis=0)
    sinf = np.concatenate([np.sin(fr), -np.sin(fr)], axis=0)
    return (np.ascontiguousarray(cosf * 0.125, dtype=np.float16),
            np.ascontiguousarray(sinf * 0.125, dtype=np.float16),
            np.ascontiguousarray(cosf, dtype=np.float16),
            np.ascontiguousarray(sinf, dtype=np.float16))


def _masks():
    p = np.arange(128)[:, None]
    c = np.arange(WMAX)[None, :]
    band = (c - p >= 1) & (c - p <= WIN)
    mw = np.where(band, 0.0, NEG).astype(BF)          # [128, 640]
    cc = np.arange(128)[None, :]
    mc = np.where(cc <= p, 0.0, NEG).astype(BF)       # [128, 128] causal
    # edge mask: cols [0,512) allowed, cols [512,640) causal triangle.
    # slicing the last w cols gives the mask for edge q-tiles (qi < 4).
    me = np.zeros((128, WMAX), dtype=BF)
    me[:, WIN:] = mc
    return mw, me


def _build_program():
    import concourse.bacc as bacc
    import concourse.tile as tile
    from concourse import mybir

    bf = mybir.dt.bfloat16
    f32 = mybir.dt.float32
    EXP = mybir.ActivationFunctionType.Exp
    RELU = mybir.ActivationFunctionType.Relu
    MULT = mybir.AluOpType.mult
    ADD = mybir.AluOpType.add
    MAX = mybir.AluOpType.max
    DIV = mybir.AluOpType.divide

    nc = bacc.Bacc("TRN2", target_bir_lowering=False, debug=False,
                   num_devices=N_CORES)

    xt_d = nc.dram_tensor("xt", [128, NKT, S], bf, kind="ExternalInput")
    wq_d = nc.dram_tensor("wq", [128, NKT, 2, 128], bf, kind="ExternalInput")
    wk_d = nc.dram_tensor("wk", [128, NKT, 128], bf, kind="ExternalInput")
    wv_d = nc.dram_tensor("wv", [128, NKT, 128], bf, kind="ExternalInput")
    wo_d = nc.dram_tensor("wo", [128, 2, D], bf, kind="ExternalInput")
    lamn_d = nc.dram_tensor("lamn", [1, 2], f32, kind="ExternalInput")
    f16 = mybir.dt.float16
    out_d = nc.dram_tensor("outp", [S, D], f16, kind="ExternalOutput")

    tqc_np, tqs_np, tkc_np, tks_np = _tables()
    mw_np, me_np = _masks()
    tqc_d = nc.inline_tensor(tqc_np, "tab_qc")
    tqs_d = nc.inline_tensor(tqs_np, "tab_qs")
    tkc_d = nc.inline_tensor(tkc_np, "tab_kc")
    tks_d = nc.inline_tensor(tks_np, "tab_ks")
    mw_d = nc.inline_tensor(mw_np, "mask_win")
    me_d = nc.inline_tensor(me_np, "mask_edge")
    idb_d = nc.inline_tensor(np.eye(128, dtype=BF), "ident_bf")
    idf_d = nc.inline_tensor(np.eye(128, dtype=np.float32), "ident_f32")

    with tile.TileContext(nc) as tc:
        with tc.tile_pool(name="xpool", bufs=1) as xp, \
             tc.tile_pool(name="wpool", bufs=1) as wp, \
             tc.tile_pool(name="pers", bufs=1) as pers:

            # DMA issue order matters: projection weights + first x chunk
            # first so matmuls start early; bulk x + tables stream in under
            # compute; phase-2/3 constants (masks, wo) last.
            wq = wp.tile([128, NKT, 2, 128], bf)
            wk = wp.tile([128, NKT, 128], bf)
            nc.sync.dma_start(out=wk[:, 0:8], in_=wk_d[:, 0:8])
            nc.sync.dma_start(out=wk[:, 8:16], in_=wk_d[:, 8:16])
            wv = wp.tile([128, NKT, 128], bf)
            nc.sync.dma_start(out=wv[:, 0:8], in_=wv_d[:, 0:8])
            nc.sync.dma_start(out=wv[:, 8:16], in_=wv_d[:, 8:16])
            idb = wp.tile([128, 128], bf)
            nc.sync.dma_start(out=idb[:], in_=idb_d[:])
            lamn = wp.tile([1, 2], f32)
            nc.sync.dma_start(out=lamn[:], in_=lamn_d[:])

            xts = []
            for nch in range(4):
                xc = xp.tile([128, NKT, 512], bf, tag=f"xt{nch}")
                xts.append(xc)
            tqc = wp.tile([128, S], f16)
            tqs = wp.tile([128, S], f16)
            tkc = wp.tile([128, S], f16)
            tks = wp.tile([128, S], f16)

            def load_chunk(nch, ndma):
                sl = slice(nch * 512, (nch + 1) * 512)
                kstep = NKT // ndma
                for i in range(ndma):
                    ksl = slice(i * kstep, (i + 1) * kstep)
                    nc.sync.dma_start(out=xts[nch][:, ksl, :],
                                      in_=xt_d[:, ksl, sl])

            for i in range(4):
                nc.sync.dma_start(out=wq[:, 4 * i:4 * i + 4],
                                  in_=wq_d[:, 4 * i:4 * i + 4])
                ksl = slice(4 * i, 4 * (i + 1))
                for j in range(2):
                    k2 = slice(4 * i + 2 * j, 4 * i + 2 * (j + 1))
                    nc.sync.dma_start(out=xts[0][:, k2, :],
                                      in_=xt_d[:, k2, 0:512])
            for i in range(2):
                sl = slice(1024 * i, 1024 * (i + 1))
                nc.sync.dma_start(out=tqc[:, sl], in_=tqc_d[:, sl])
                nc.sync.dma_start(out=tqs[:, sl], in_=tqs_d[:, sl])
                nc.sync.dma_start(out=tkc[:, sl], in_=tkc_d[:, sl])
                nc.sync.dma_start(out=tks[:, sl], in_=tks_d[:, sl])
            load_chunk(1, 8)
            load_chunk(2, 4)
            load_chunk(3, 4)
            mw = wp.tile([128, WMAX], bf)
            nc.sync.dma_start(out=mw[:], in_=mw_d[:])
            me = wp.tile([128, WMAX], bf)
            nc.sync.dma_start(out=me[:], in_=me_d[:])
            wo = wp.tile([128, 2, D], bf)
            for i in range(4):
                nc.sync.dma_start(out=wo[:, :, 512 * i:512 * (i + 1)],
                                  in_=wo_d[:, :, 512 * i:512 * (i + 1)])
            lamb = wp.tile([128, 2], f32)
            nc.gpsimd.partition_broadcast(lamb[:], lamn[:])

            # q stored zero-padded to full 128 contraction rows per half:
            # qtp0 rows 0:64 hold half-0 q, rows 64:128 are zero; qtp1 is
            # the mirror. A 128-contraction matmul runs at 2x the column
            # rate of a 64-contraction one, so the padded zeros are free.
            qtp0 = pers.tile([128, 2, S], bf)
            qtp1 = pers.tile([128, 2, S], bf)
            kt = pers.tile([128, S], bf)         # RoPE'd k, hd-major
            vsm = pers.tile([128, NQT, 128], bf)  # v, S-major [s, hd]
            att = pers.tile([128, 2, S], bf)     # attention out^T, hd-major
            zeros = pers.tile([128, WMAX], bf)
            nc.gpsimd.memset(qtp0[64:128, :, :], 0.0)
            nc.gpsimd.memset(qtp1[0:64, :, :], 0.0)
            nc.gpsimd.memset(zeros[:], 0.0)

            # ---- Phase 1: projections + RoPE + v transpose ----
            with tc.tile_pool(name="pp", bufs=1, space="PSUM") as pp, \
                 tc.tile_pool(name="pt", bufs=2) as pt:
                for nch in range(4):
                    sl = slice(nch * 512, (nch + 1) * 512)
                    ps_q0 = pp.tile([128, 512], f32, tag="pq0", bufs=2)
                    ps_q1 = pp.tile([128, 512], f32, tag="pq1", bufs=2)
                    ps_k = pp.tile([128, 512], f32, tag="pk", bufs=1)
                    ps_v = pp.tile([128, 512], f32, tag="pv", bufs=1)
                    for kti in range(NKT):
                        st = kti == 0
                        sp = kti == NKT - 1
                        rhs = xts[nch][:, kti, :]
                        nc.tensor.matmul(ps_q0[:], wq[:, kti, 0, :], rhs, start=st, stop=sp)
                        nc.tensor.matmul(ps_q1[:], wq[:, kti, 1, :], rhs, start=st, stop=sp)
                        nc.tensor.matmul(ps_k[:], wk[:, kti, :], rhs, start=st, stop=sp)
                        nc.tensor.matmul(ps_v[:], wv[:, kti, :], rhs, start=st, stop=sp)
                    for ps, outlo, outhi, tabc, tabs in (
                            (ps_q0, qtp0[0:64, 0, sl], qtp1[64:128, 0, sl], tqc, tqs),
                            (ps_q1, qtp0[0:64, 1, sl], qtp1[64:128, 1, sl], tqc, tqs),
                            (ps_k, None, None, tkc, tks)):
                        # out = ps*cos + rot_half(ps)*sin, sign folded in tabs
                        m1 = pt.tile([128, 512], f32, tag="m1")
                        m2 = pt.tile([128, 512], f32, tag="m2")
                        nc.vector.tensor_mul(m1[:], ps[:], tabc[:, sl])
                        nc.vector.tensor_mul(m2[0:64, :], ps[64:128, :], tabs[64:128, sl])
                        nc.vector.tensor_mul(m2[64:128, :], ps[0:64, :], tabs[0:64, sl])
                        if outlo is None:
                            nc.vector.tensor_add(kt[:, sl], m1[:], m2[:])
                        else:
                            nc.vector.tensor_add(outlo, m1[0:64, :], m2[0:64, :])
                            nc.vector.tensor_add(outhi, m1[64:128, :], m2[64:128, :])
                    vtmp = pt.tile([128, 512], bf, tag="vtmp")
                    nc.vector.tensor_copy(out=vtmp[:], in_=ps_v[:])
                    ps_tv = pp.tile([128, 4, 128], bf, tag="ptv", bufs=2)
                    for j in range(4):
                        nc.tensor.transpose(ps_tv[:, j, :], vtmp[:, 128 * j:128 * (j + 1)], idb[:])
                    nc.vector.tensor_copy(out=vsm[:, 4 * nch:4 * (nch + 1), :], in_=ps_tv[:])

            # ---- Phase 2: attention ----
            with tc.tile_pool(name="psc", bufs=1, space="PSUM") as psc, \
                 tc.tile_pool(name="pse", bufs=1) as pse, \
                 tc.tile_pool(name="psm", bufs=1) as psm:
                for qi in range(NQT):
                    qsl = slice(qi * 128, (qi + 1) * 128)
                    kw = min(qi + 1, 5)
                    w = kw * 128
                    kstart = max(0, qi - 4)
                    kwin = slice(kstart * 128, kstart * 128 + w)

                    # both heads' AV share one 256-wide PSUM tile
                    ps_av = psc.tile([128, 2, 128], f32, tag="av", bufs=1)
                    gts = pse.tile([128, 2, 5, 128], bf, tag="gts", bufs=2)

                    for h in range(2):
                        ps_s1 = psc.tile([128, WMAX], f32, tag="s", bufs=2)
                        ps_s2 = psc.tile([128, WMAX], f32, tag="s", bufs=2)
                        for ps, lhsq in ((ps_s1, qtp0), (ps_s2, qtp1)):
                            lhs = lhsq[:, h, qsl]
                            if qi >= 4:
                                # causal edge lives only in block 0, so the
                                # mask matmul covers 128 cols; the score
                                # matmul splits at that boundary
                                nc.tensor.matmul(ps[:, 0:128], idb[:], mw[:, 0:128],
                                                 start=True, stop=False)
                                nc.tensor.matmul(ps[:, 0:128], lhs,
                                                 kt[:, kwin][:, 0:128],
                                                 start=False, stop=True)
                                nc.tensor.matmul(ps[:, 128:512], lhs,
                                                 kt[:, kwin][:, 128:512],
                                                 start=True, stop=True)
                                nc.tensor.matmul(ps[:, 512:640], idb[:],
                                                 mw[:, 512:640],
                                                 start=True, stop=False)
                                nc.tensor.matmul(ps[:, 512:640], lhs,
                                                 kt[:, kwin][:, 512:640],
                                                 start=False, stop=True)
                            else:
                                nc.tensor.matmul(ps[:, 0:w], idb[:],
                                                 me[:, WMAX - w:WMAX],
                                                 start=True, stop=False)
                                nc.tensor.matmul(ps[:, 0:w], lhs,
                                                 kt[:, kwin][:, 0:w],
                                                 start=False, stop=True)

                        e1 = pse.tile([128, WMAX], bf, tag="e1", bufs=2)
                        e2 = pse.tile([128, WMAX], bf, tag="e2", bufs=2)
                        s1 = psm.tile([128, 1], f32, tag="s1", bufs=4)
                        s2 = psm.tile([128, 1], f32, tag="s2", bufs=4)
                        nc.scalar.activation(out=e1[:, 0:w], in_=ps_s1[:, 0:w],
                                             func=EXP, accum_out=s1[:])
                        nc.scalar.activation(out=e2[:, 0:w], in_=ps_s2[:, 0:w],
                                             func=EXP, accum_out=s2[:])

                        # cneg = -(lam * s1 / s2)   (lamn holds -lam)
                        r2 = psm.tile([128, 1], f32, tag="r2", bufs=4)
                        nc.vector.reciprocal(out=r2[:], in_=s2[:])
                        cneg = psm.tile([128, 1], f32, tag="cneg", bufs=4)
                        nc.vector.scalar_tensor_tensor(
                            out=cneg[:], in0=s1[:], scalar=lamb[:, h:h + 1],
                            in1=r2[:], op0=MULT, op1=MULT)
                        # g0 = e1 + cneg*e2 ; g = relu(g0), accum D'
                        g0 = pse.tile([128, WMAX], bf, tag="g0", bufs=2)
                        g = pse.tile([128, WMAX], bf, tag="g", bufs=2)
                        dsum = psm.tile([128, 1], f32, tag="dsum", bufs=4)
                        nc.vector.scalar_tensor_tensor(
                            out=g0[:, 0:w], in0=e2[:, 0:w], scalar=cneg[:],
                            in1=e1[:, 0:w], op0=MULT, op1=ADD)
                        if h == 0:
                            nc.scalar.activation(
                                out=g[:, 0:w], in_=g0[:, 0:w], func=RELU,
                                accum_out=dsum[:])
                        else:
                            nc.vector.tensor_scalar(
                                out=g[:, 0:w], in0=g0[:, 0:w], scalar1=0.0,
                                scalar2=0.0, op0=MAX, op1=ADD, accum_out=dsum[:])
                        # recd = 1 / (D' + 1e-6 * s1); gn = g * recd
                        dtmp = psm.tile([128, 1], f32, tag="dtmp", bufs=4)
                        nc.vector.scalar_tensor_tensor(
                            out=dtmp[:], in0=s1[:], scalar=1e-6, in1=dsum[:],
                            op0=MULT, op1=ADD)
                        recd = psm.tile([128, 1], f32, tag="recd", bufs=4)
                        nc.vector.reciprocal(out=recd[:], in_=dtmp[:])
                        gn = pse.tile([128, WMAX], bf, tag="gn", bufs=2)
                        nc.vector.tensor_scalar(
                            out=gn[:, 0:w], in0=g[:, 0:w], scalar1=recd[:],
                            scalar2=0.0, op0=MULT, op1=ADD)

                        # transpose gn -> gT (PSUM) -> SBUF
                        ps_tr = psc.tile([128, kw, 128], bf, tag="trg", bufs=2)
                        for j in range(kw):
                            nc.tensor.transpose(ps_tr[:, j, :],
                                                gn[:, 128 * j:128 * (j + 1)], idb[:])
                        if h == 0:
                            nc.vector.tensor_copy(out=gts[:, 0, 0:kw, :], in_=ps_tr[:])
                        else:
                            nc.scalar.copy(out=gts[:, 1, 0:kw, :], in_=ps_tr[:])

                    # AV for both heads at once: [k,hd]^T-contract x [k, 2*128]
                    for j in range(kw):
                        nc.tensor.matmul(ps_av[:], vsm[:, kstart + j, :],
                                         gts[:, :, j, :],
                                         start=(j == 0), stop=(j == kw - 1))

                    nc.vector.tensor_copy(out=att[:, :, qsl], in_=ps_av[:])

                    # out-projection for this q-tile, interleaved so the PE
                    # fills attention bubbles and the output DMA spreads out
                    so = pse.tile([128, 2048], f16, tag="so", bufs=2)
                    for dch in range(4):
                        dsl = slice(dch * 512, (dch + 1) * 512)
                        ps_o = psc.tile([128, 512], f32, tag="o", bufs=1)
                        nc.tensor.matmul(ps_o[:], att[:, 0, qsl], wo[:, 0, dsl],
                                         start=True, stop=False)
                        nc.tensor.matmul(ps_o[:], att[:, 1, qsl], wo[:, 1, dsl],
                                         start=False, stop=True)
                        if dch % 2 == 0:
                            nc.vector.tensor_copy(out=so[:, dsl], in_=ps_o[:])
                        else:
                            nc.scalar.copy(out=so[:, dsl], in_=ps_o[:])
                        if dch % 2 == 1:
                            dsl2 = slice((dch - 1) * 512, (dch + 1) * 512)
                            nc.sync.dma_start(out=out_d[qsl, dsl2], in_=so[:, dsl2])

    nc.compile()
    return nc


def get_program():
    if "nc" not in _CACHE:
        _CACHE["nc"] = _build_program()
    return _CACHE["nc"]


def _prep_inputs(x, Wq, Wk, Wv, Wo, lam):
    xt = np.ascontiguousarray(x.reshape(S, D).T.astype(BF)
                              .reshape(NKT, 128, S).transpose(1, 0, 2))
    in_maps = []
    for c in range(N_CORES):
        h0 = 2 * c
        kv = c // 2
        wq_c = np.ascontiguousarray(
            Wq[:, h0 * 128:(h0 + 2) * 128].astype(BF)
            .reshape(NKT, 128, 2, 128).transpose(1, 0, 2, 3))
        wk_c = np.ascontiguousarray(
            Wk[:, kv * 128:(kv + 1) * 128].astype(BF)
            .reshape(NKT, 128, 128).transpose(1, 0, 2))
        wv_c = np.ascontiguousarray(
            Wv[:, kv * 128:(kv + 1) * 128].astype(BF)
            .reshape(NKT, 128, 128).transpose(1, 0, 2))
        wo_c = np.ascontiguousarray(
            Wo[h0 * 128:(h0 + 2) * 128, :].astype(BF)
            .reshape(2, 128, D).transpose(1, 0, 2))
        lamn_c = np.array([[-float(lam[h0]), -float(lam[h0 + 1])]], dtype=np.float32)
        in_maps.append({"xt": xt, "wq": wq_c, "wk": wk_c, "wv": wv_c,
                        "wo": wo_c, "lamn": lamn_c})
    return in_maps


def kernel(x, Wq, Wk, Wv, Wo, lam):
    from concourse.bass_utils import run_bass_kernel_spmd

    nc = get_program()
    in_maps = _prep_inputs(np.asarray(x), np.asarray(Wq), np.asarray(Wk),
                           np.asarray(Wv), np.asarray(Wo), np.asarray(lam))
    res = run_bass_kernel_spmd(nc, in_maps, list(range(N_CORES)))
    out = np.zeros((S, D), dtype=np.float32)
    for c in range(N_CORES):
        out += res.results[c]["outp"].astype(np.float32)
    return out.reshape(1, S, D)

